# revision 1
# baseline (speedup 1.0000x reference)
"""LSEP loss kernel for Trainium2 (8 NeuronCores, data-parallel on batch).

loss = log1p( sum_b [ (sum_{c: t=0} e^{x_bc}) * (sum_{c: t=1} e^{-x_bc}) ] ) / B

Encoding: host ships x as bf16 and the target as m16 = t<<15 (uint16, the
bf16 sign-bit position). On device, z = x XOR m16 computes x*(1-2t), so ONE
exp pass yields e^x for negatives and e^{-x} for positives. With
S = sum_c e^z and D = sum_c (e^z XOR m16) = neg_sum - pos_sum, the per-row
product is 4*neg*pos = (S+D)(S-D).

Row sums: pair-halving tensor_tensor adds (the 2x DVE mode; tensor_reduce
and scalar_tensor_tensor both measured 1x on HW, and flat contiguous APs —
multi-dim strided views cost ~15%) down to 6 wide, then one 1x
tensor_reduce to f32 row scalars; ACT squares them with accum_out so the
per-tile sums of S^2/D^2 cost DVE nothing. Everything runs on DVE+ACT:
bitwise ops are DVE-only, and offloading tree stages to GPSIMD measured
slower (shared SBUF port, optimistic Pool cost model in the scheduler).
Engines execute their queues in order, so the loop is software-pipelined:
each tile's ACT-dependent ops (se-xor, trees) are emitted one tile after
its DMA/z-xor/exp. x and m16 ship interleaved in one [rows, 48] uint16
tensor (one DMA per tile).

Output: [128,2] per-core partials (sum of S^2, sum of D^2); host computes
(sum_S2 - sum_D2)/4 and applies log1p (the gather/unshard step).
"""

import numpy as np

B = 2_000_000
C = 24
NCORES = 8
P = 128
# half-size leading tiles shorten the pipeline fill ramp; steady-state
# tiles stay at the measured-optimal 196 rows per partition
KS = (98, 98) + (196,) * 9
TILES = len(KS)
RPC_RAW = B // NCORES            # 250_000 real rows per core
RPC = P * sum(KS)                # 250_880 padded rows per core

_cached = {}


def _build(rows, ks):
    from contextlib import ExitStack

    import concourse.bacc as bacc
    import concourse.tile as tile
    from concourse import mybir

    f32 = mybir.dt.float32
    bf16 = mybir.dt.bfloat16
    u16 = mybir.dt.uint16
    Alu = mybir.AluOpType
    Act = mybir.ActivationFunctionType
    X = mybir.AxisListType.X
    XY = mybir.AxisListType.XY

    nc = bacc.Bacc("TRN2", debug=False, num_devices=NCORES)
    xm = nc.dram_tensor("xm", [rows, 2 * C], u16, kind="ExternalInput").ap()
    out = nc.dram_tensor("o", [P, 2], f32, kind="ExternalOutput").ap()

    tiles = len(ks)
    xmv = []
    base = 0
    for ki in ks:
        xmv.append(
            xm[base : base + P * ki, :].rearrange("(p k) c -> p k c", p=P, k=ki)
        )
        base += P * ki

    with tile.TileContext(nc) as tc, ExitStack() as ctx:
        io = ctx.enter_context(tc.tile_pool(name="io", bufs=4))
        ep = ctx.enter_context(tc.tile_pool(name="ep", bufs=4))
        tp = ctx.enter_context(tc.tile_pool(name="tp", bufs=4))
        accp = ctx.enter_context(tc.tile_pool(name="accp", bufs=1))
        accS = accp.tile([P, tiles], f32)  # per-tile sum of S^2 rows
        accD = accp.tile([P, tiles], f32)  # per-tile sum of D^2 rows
        V = nc.vector
        G = nc.gpsimd

        st = {}  # per-tile tiles carried across pipeline stages

        def stage_a(i):
            # one DMA for interleaved [x | m16] rows; z = x ^ m; exp on ACT
            ki = ks[i]
            xmt = io.tile([P, ki, 2 * C], u16, tag="xm")
            nc.sync.dma_start(out=xmt, in_=xmv[i])
            xu = xmt[:, :, 0:C]
            mt = xmt[:, :, C : 2 * C]
            V.tensor_tensor(out=xu, in0=xu, in1=mt, op=Alu.bitwise_xor)
            e = ep.tile([P, ki, C], bf16, tag="e")
            nc.scalar.activation(out=e, in_=xu.bitcast(bf16), func=Act.Exp)
            st[i] = {"e": e, "m": mt}

        def stage_b(i):
            # se = e ^ m (in place over m); both trees on DVE
            ki = ks[i]
            e, mt = st[i]["e"], st[i]["m"]
            se = mt.bitcast(bf16)
            V.tensor_tensor(out=mt, in0=e.bitcast(u16), in1=mt,
                            op=Alu.bitwise_xor)
            s1 = tp.tile([P, ki, 12], bf16, tag="s1")
            V.tensor_add(s1, e[:, :, 0:12], e[:, :, 12:24])
            s2 = tp.tile([P, ki, 6], bf16, tag="s2")
            V.tensor_add(s2, s1[:, :, 0:6], s1[:, :, 6:12])
            sS = tp.tile([P, ki], f32, tag="sS")
            V.tensor_reduce(out=sS, in_=s2, axis=X, op=Alu.add)
            sq = tp.tile([P, ki], f32, tag="sq")
            nc.scalar.activation(out=sq, in_=sS, func=Act.Square,
                                 accum_out=accS[:, i : i + 1])
            d1 = tp.tile([P, ki, 12], bf16, tag="d1")
            V.tensor_add(d1, se[:, :, 0:12], se[:, :, 12:24])
            d2 = tp.tile([P, ki, 6], bf16, tag="d2")
            V.tensor_add(d2, d1[:, :, 0:6], d1[:, :, 6:12])
            dS = tp.tile([P, ki], f32, tag="dS")
            V.tensor_reduce(out=dS, in_=d2, axis=X, op=Alu.add)
            dq = tp.tile([P, ki], f32, tag="dq")
            nc.scalar.activation(out=dq, in_=dS, func=Act.Square,
                                 accum_out=accD[:, i : i + 1])
            del st[i]

        stage_a(0)
        stage_a(1)
        stage_b(0)
        for i in range(tiles):
            if i + 2 < tiles:
                stage_a(i + 2)
            if i + 1 < tiles:
                stage_b(i + 1)
        a1 = accp.tile([P, 2], f32)
        nc.vector.tensor_reduce(out=a1[:, 0:1], in_=accS, axis=X, op=Alu.add)
        nc.vector.tensor_reduce(out=a1[:, 1:2], in_=accD, axis=X, op=Alu.add)
        nc.sync.dma_start(out=out, in_=a1)
    nc.compile()
    return nc


def _get_nc():
    key = (RPC, KS)
    if key not in _cached:
        _cached[key] = _build(RPC, KS)
    return _cached[key]


def _f32_to_bf16_u16(a):
    # round-to-nearest-even f32 -> bf16, as uint16 bit pattern
    u = a.view(np.uint32)
    r = ((u >> 16) & 1) + np.uint32(0x7FFF)
    return ((u + r) >> 16).astype(np.uint16)


def _shard(input, target):
    xb = _f32_to_bf16_u16(input)
    mb = (target << 15).astype(np.uint16)
    in_maps = []
    for c in range(NCORES):
        xs = np.zeros((RPC, 2 * C), np.uint16)
        xs[:RPC_RAW, 0:C] = xb[c * RPC_RAW : (c + 1) * RPC_RAW]
        xs[:RPC_RAW, C : 2 * C] = mb[c * RPC_RAW : (c + 1) * RPC_RAW]
        in_maps.append({"xm": xs})
    return in_maps


_last_results = None


def kernel(input, target):
    global _last_results
    input = np.ascontiguousarray(np.asarray(input, dtype=np.float32))
    target = np.ascontiguousarray(np.asarray(target, dtype=np.int32))
    assert input.shape == (B, C) and target.shape == (B, C)

    from concourse.bass_utils import run_bass_kernel_spmd

    nc = _get_nc()
    in_maps = _shard(input, target)
    res = run_bass_kernel_spmd(nc, in_maps, core_ids=list(range(NCORES)))
    _last_results = res
    ssum = float(np.sum([r["o"][:, 0] for r in res.results], dtype=np.float64))
    dsum = float(np.sum([r["o"][:, 1] for r in res.results], dtype=np.float64))
    total = (ssum - dsum) / 4.0
    return np.asarray(np.log1p(total) / B, dtype=np.float32)



# revision 2
# speedup vs baseline: 1.1947x; 1.1947x over previous
"""LSEP loss kernel V4 for Trainium2 (8 NeuronCores, data-parallel on batch).

loss = log1p( sum_b [ (sum_{c: t=0} e^{x_bc}) * (sum_{c: t=1} e^{-x_bc}) ] ) / B

V4 over V3 (structure identical: one fp8 stream with the target bit in the
LSB, transposed [120, M] layout, sign-mask + XOR -> +-e, S|D row sums on the
PE, bn_stats drain, host finish):
  - exp / sign-mask / XOR run at 2-round granularity (4096-col blocks) to
    amortize per-instruction overheads,
  - bn_stats for round r is emitted one iteration later so the DVE never
    stalls waiting on the PE (psum pool holds 4 banks),
  - a 256/4096 column slice of each block computes exp on the DVE instead
    of ACT via the Schraudolph bit trick in the fp8 domain:
    bits(e^z) ~= round(8/ln2 * z + 55.6), one tensor_scalar (mult, add)
    with u8 saturation -- shaves the ACT ceiling, costs idle DVE slack,
  - host clips z to [-15, 4.4] so every e^z is finite in fp8 on both paths.
"""

import numpy as np
import ml_dtypes

B = 2_000_000
C = 24
NCORES = 8
G = 5                       # row groups packed on partitions
K = G * C                   # 120 live partitions
SUB = 512                   # batch rows per matmul (PSUM bank free size)
RPS = 4                     # sub-tiles per round (4 psum column positions)
ROUNDS = 25
M = ROUNDS * RPS * SUB      # 51_200 columns per group per core
RPC = G * M                 # 256_000 padded rows per core
RPC_RAW = B // NCORES       # 250_000 real rows per core

BLK = 2                     # rounds per block
NBLK = (ROUNDS + 1) // 2    # 13 (last block single round)
SCH = 256                   # Schraudolph columns per full block (on DVE)
A_SCH = 8.0 / np.log(2.0)
B_SCH = 55.6

_cached = {}


def _build():
    from contextlib import ExitStack

    import concourse.bacc as bacc
    import concourse.tile as tile
    from concourse import mybir

    f32 = mybir.dt.float32
    fp8 = mybir.dt.float8e4
    u8 = mybir.dt.uint8
    u16 = mybir.dt.uint16
    Alu = mybir.AluOpType
    Act = mybir.ActivationFunctionType

    nc = bacc.Bacc("TRN2", debug=False, num_devices=NCORES)
    zd = nc.dram_tensor("z", [K, M], u8, kind="ExternalInput").ap()
    wd = nc.dram_tensor("w", [K, 2 * 2 * G], u8, kind="ExternalInput").ap()
    out = nc.dram_tensor("o", [128, 6 * ROUNDS], f32, kind="ExternalOutput").ap()

    RW = RPS * SUB   # 2048 columns per round
    BW = BLK * RW    # 4096 columns per block

    def blk_w(bk):
        return BW if 2 * bk + 1 < ROUNDS else RW

    with tile.TileContext(nc) as tc, ExitStack() as ctx:
        io = ctx.enter_context(tc.tile_pool(name="io", bufs=3))
        mm = ctx.enter_context(tc.tile_pool(name="mm", bufs=3))
        ee = ctx.enter_context(tc.tile_pool(name="ee", bufs=3))
        ps = ctx.enter_context(tc.psum_pool(name="ps", bufs=4))
        ap_ = ctx.enter_context(tc.tile_pool(name="ac", bufs=1))
        V = nc.vector

        wt = ap_.tile([K, 2 * 2 * G], u8)
        nc.sync.dma_start(out=wt, in_=wd)
        w_s = wt[:, 0 : 2 * G].bitcast(fp8)        # (+1 | 0) block-diag
        w_d = wt[:, 2 * G : 4 * G].bitcast(fp8)    # ( 0 | +1) block-diag
        acc = ap_.tile([128, 6 * ROUNDS], f32)

        zb, eb, mb = {}, {}, {}

        def stage_a(bk):
            if bk >= NBLK:
                return
            w_ = blk_w(bk)
            zt = io.tile([K, BW], u8, tag="z")
            nc.sync.dma_start(out=zt[:, 0:w_], in_=zd[:, bk * BW : bk * BW + w_])
            zb[bk] = zt

        def stage_b(bk):
            if bk >= NBLK:
                return
            w_ = blk_w(bk)
            zt = zb[bk]
            e8 = ee.tile([K, BW], fp8, tag="e")
            sch = SCH if w_ == BW else 0
            nc.scalar.activation(
                out=e8[:, 0 : w_ - sch],
                in_=zt[:, 0 : w_ - sch].bitcast(fp8),
                func=Act.Exp,
            )
            if sch:
                V.tensor_scalar(
                    out=e8[:, w_ - sch : w_].bitcast(u8),
                    in0=zt[:, w_ - sch : w_].bitcast(fp8),
                    scalar1=A_SCH,
                    scalar2=B_SCH,
                    op0=Alu.mult,
                    op1=Alu.add,
                )
            mp = mm.tile([K, BW], u8, tag="mp")
            V.tensor_scalar(
                out=mp[:, 0:w_].bitcast(u16),
                in0=zt[:, 0:w_].bitcast(u16),
                scalar1=7,
                scalar2=0x8080,
                op0=Alu.logical_shift_left,
                op1=Alu.bitwise_and,
            )
            eb[bk] = e8
            mb[bk] = mp

        def stage_c(bk):
            if bk >= NBLK:
                return
            w_ = blk_w(bk)
            e8, mp = eb[bk], mb[bk]
            V.tensor_tensor(
                out=mp[:, 0:w_].bitcast(u16),
                in0=e8[:, 0:w_].bitcast(u16),
                in1=mp[:, 0:w_].bitcast(u16),
                op=Alu.bitwise_xor,
            )

        sdt = {}

        def stage_d(r):
            bk, off = divmod(r, BLK)
            e8 = eb[bk][:, off * RW : (off + 1) * RW]
            se = mb[bk][:, off * RW : (off + 1) * RW].bitcast(fp8)
            sd = ps.tile([128, SUB], f32, tag="sd")
            if r < 4:
                V.memset(sd, 0.0)
            for s in range(RPS):
                pos = 32 * s
                sl = slice(s * SUB, (s + 1) * SUB)
                nc.tensor.matmul(
                    out=sd[pos : pos + 2 * G, :],
                    lhsT=w_s,
                    rhs=e8[:, sl],
                    start=True,
                    stop=False,
                    tile_position=(0, pos),
                )
                nc.tensor.matmul(
                    out=sd[pos : pos + 2 * G, :],
                    lhsT=w_d,
                    rhs=se[:, sl],
                    start=False,
                    stop=True,
                    tile_position=(0, pos),
                )
            sdt[r] = sd

        def stage_e(r):
            if r < 0 or r not in sdt:
                return
            V.bn_stats(
                out=acc[0 : 96 + 2 * G, 6 * r : 6 * (r + 1)],
                in_=sdt.pop(r)[0 : 96 + 2 * G, :],
            )

        stage_a(0)
        stage_a(1)
        stage_b(0)
        for bk in range(NBLK):
            stage_a(bk + 2)
            stage_b(bk + 1)
            stage_c(bk)
            for r in range(bk * BLK, min((bk + 1) * BLK, ROUNDS)):
                stage_d(r)
                stage_e(r - 1)
        stage_e(ROUNDS - 1)
        nc.sync.dma_start(out=out[0 : 96 + 2 * G, :], in_=acc[0 : 96 + 2 * G, :])
    nc.compile()
    return nc


def _get_nc():
    if "nc" not in _cached:
        _cached["nc"] = _build()
    return _cached["nc"]


def _shard(input, target):
    fp8 = ml_dtypes.float8_e4m3
    z = np.where(target > 0, -input, input)
    np.clip(z, -15.0, 4.4, out=z)
    z8 = z.astype(fp8).view(np.uint8)
    z8 = (z8 & np.uint8(0xFE)) | (target > 0).astype(np.uint8)
    bd = np.zeros((K, G), np.float32)
    for g in range(G):
        bd[g * C : (g + 1) * C, g] = 1.0
    zero = np.zeros_like(bd)
    w = np.concatenate([bd, zero, zero, bd], axis=1)  # w_s (+1|0), w_d (0|+1)
    w8 = w.astype(fp8).view(np.uint8)
    in_maps = []
    for cr in range(NCORES):
        zp = np.zeros((RPC, C), np.uint8)
        zp[:RPC_RAW] = z8[cr * RPC_RAW : (cr + 1) * RPC_RAW]
        zt = zp.reshape(G, M, C).transpose(0, 2, 1).reshape(K, M)
        in_maps.append({"z": np.ascontiguousarray(zt), "w": w8})
    return in_maps


_last_results = None


def kernel(input, target):
    global _last_results
    input = np.ascontiguousarray(np.asarray(input, dtype=np.float32))
    target = np.ascontiguousarray(np.asarray(target, dtype=np.int32))
    assert input.shape == (B, C) and target.shape == (B, C)

    from concourse.bass_utils import run_bass_kernel_spmd

    nc = _get_nc()
    in_maps = _shard(input, target)
    res = run_bass_kernel_spmd(nc, in_maps, core_ids=list(range(NCORES)))
    _last_results = res

    sign = np.zeros(128, np.float64)
    for s in range(RPS):
        sign[32 * s : 32 * s + G] = 1.0
        sign[32 * s + G : 32 * s + 2 * G] = -1.0

    total = 0.0
    for r in res.results:
        a = r["o"].astype(np.float64).reshape(128, ROUNDS, 6)
        sumsq = a[..., 2] + a[..., 0] * a[..., 1] ** 2 \
            + a[..., 5] + a[..., 3] * a[..., 4] ** 2
        total += float(np.einsum("p,pr->", sign, sumsq))
    total /= 4.0
    return np.asarray(np.log1p(total) / B, dtype=np.float32)


# revision 3
# speedup vs baseline: 1.2012x; 1.0054x over previous
"""LSEP loss kernel V4 for Trainium2 (8 NeuronCores, data-parallel on batch).

loss = log1p( sum_b [ (sum_{c: t=0} e^{x_bc}) * (sum_{c: t=1} e^{-x_bc}) ] ) / B

V4 over V3 (structure identical: one fp8 stream with the target bit in the
LSB, transposed [120, M] layout, sign-mask + XOR -> +-e, S|D row sums on the
PE, bn_stats drain, host finish):
  - exp / sign-mask / XOR run at 2-round granularity (4096-col blocks) to
    amortize per-instruction overheads,
  - bn_stats for round r is emitted one iteration later so the DVE never
    stalls waiting on the PE (psum pool holds 4 banks),
  - a 256/4096 column slice of each block computes exp on the DVE instead
    of ACT via the Schraudolph bit trick in the fp8 domain:
    bits(e^z) ~= round(8/ln2 * z + 55.6), one tensor_scalar (mult, add)
    with u8 saturation -- shaves the ACT ceiling, costs idle DVE slack,
  - host clips z to [-15, 4.4] so every e^z is finite in fp8 on both paths.
"""

import numpy as np
import ml_dtypes

B = 2_000_000
C = 24
NCORES = 8
G = 5                       # row groups packed on partitions
K = G * C                   # 120 live partitions
SUB = 512                   # batch rows per matmul (PSUM bank free size)
RPS = 4                     # sub-tiles per round (4 psum column positions)
ROUNDS = 25
M = ROUNDS * RPS * SUB      # 51_200 columns per group per core
RPC = G * M                 # 256_000 padded rows per core
RPC_RAW = B // NCORES       # 250_000 real rows per core

BLK = 2                     # rounds per block
NBLK = (ROUNDS + 1) // 2    # 13 (last block single round)
SCH = 384                   # Schraudolph columns per full block (on DVE)
A_SCH = 8.0 / np.log(2.0)
B_SCH = 55.6

_cached = {}


def _build():
    from contextlib import ExitStack

    import concourse.bacc as bacc
    import concourse.tile as tile
    from concourse import mybir

    f32 = mybir.dt.float32
    fp8 = mybir.dt.float8e4
    u8 = mybir.dt.uint8
    u16 = mybir.dt.uint16
    Alu = mybir.AluOpType
    Act = mybir.ActivationFunctionType

    nc = bacc.Bacc("TRN2", debug=False, num_devices=NCORES)
    zd = nc.dram_tensor("z", [K, M], u8, kind="ExternalInput").ap()
    wd = nc.dram_tensor("w", [K, 2 * 2 * G], u8, kind="ExternalInput").ap()
    out = nc.dram_tensor("o", [128, 6 * ROUNDS], f32, kind="ExternalOutput").ap()

    RW = RPS * SUB   # 2048 columns per round
    BW = BLK * RW    # 4096 columns per block

    def blk_w(bk):
        return BW if 2 * bk + 1 < ROUNDS else RW

    with tile.TileContext(nc) as tc, ExitStack() as ctx:
        io = ctx.enter_context(tc.tile_pool(name="io", bufs=3))
        mm = ctx.enter_context(tc.tile_pool(name="mm", bufs=3))
        ee = ctx.enter_context(tc.tile_pool(name="ee", bufs=3))
        ps = ctx.enter_context(tc.psum_pool(name="ps", bufs=4))
        ap_ = ctx.enter_context(tc.tile_pool(name="ac", bufs=1))
        V = nc.vector

        wt = ap_.tile([K, 2 * 2 * G], u8)
        nc.sync.dma_start(out=wt, in_=wd)
        w_s = wt[:, 0 : 2 * G].bitcast(fp8)        # (+1 | 0) block-diag
        w_d = wt[:, 2 * G : 4 * G].bitcast(fp8)    # ( 0 | +1) block-diag
        acc = ap_.tile([128, 6 * ROUNDS], f32)

        zb, eb, mb = {}, {}, {}

        def stage_a(bk):
            if bk >= NBLK:
                return
            w_ = blk_w(bk)
            zt = io.tile([K, BW], u8, tag="z")
            nc.sync.dma_start(out=zt[:, 0:w_], in_=zd[:, bk * BW : bk * BW + w_])
            zb[bk] = zt

        def stage_b(bk):
            if bk >= NBLK:
                return
            w_ = blk_w(bk)
            zt = zb[bk]
            e8 = ee.tile([K, BW], fp8, tag="e")
            sch = SCH if w_ == BW else 0
            nc.scalar.activation(
                out=e8[:, 0 : w_ - sch],
                in_=zt[:, 0 : w_ - sch].bitcast(fp8),
                func=Act.Exp,
            )
            if sch:
                V.tensor_scalar(
                    out=e8[:, w_ - sch : w_].bitcast(u8),
                    in0=zt[:, w_ - sch : w_].bitcast(fp8),
                    scalar1=A_SCH,
                    scalar2=B_SCH,
                    op0=Alu.mult,
                    op1=Alu.add,
                )
            mp = mm.tile([K, BW], u8, tag="mp")
            V.tensor_scalar(
                out=mp[:, 0:w_].bitcast(u16),
                in0=zt[:, 0:w_].bitcast(u16),
                scalar1=7,
                scalar2=0x8080,
                op0=Alu.logical_shift_left,
                op1=Alu.bitwise_and,
            )
            eb[bk] = e8
            mb[bk] = mp

        def stage_c(bk):
            if bk >= NBLK:
                return
            w_ = blk_w(bk)
            e8, mp = eb[bk], mb[bk]
            V.tensor_tensor(
                out=mp[:, 0:w_].bitcast(u16),
                in0=e8[:, 0:w_].bitcast(u16),
                in1=mp[:, 0:w_].bitcast(u16),
                op=Alu.bitwise_xor,
            )

        sdt = {}

        def stage_d(r):
            bk, off = divmod(r, BLK)
            e8 = eb[bk][:, off * RW : (off + 1) * RW]
            se = mb[bk][:, off * RW : (off + 1) * RW].bitcast(fp8)
            sd = ps.tile([128, SUB], f32, tag="sd")
            if r < 4:
                V.memset(sd, 0.0)
            for s in range(RPS):
                pos = 32 * s
                sl = slice(s * SUB, (s + 1) * SUB)
                nc.tensor.matmul(
                    out=sd[pos : pos + 2 * G, :],
                    lhsT=w_s,
                    rhs=e8[:, sl],
                    start=True,
                    stop=False,
                    tile_position=(0, pos),
                )
                nc.tensor.matmul(
                    out=sd[pos : pos + 2 * G, :],
                    lhsT=w_d,
                    rhs=se[:, sl],
                    start=False,
                    stop=True,
                    tile_position=(0, pos),
                )
            sdt[r] = sd

        def stage_e(r):
            if r < 0 or r not in sdt:
                return
            V.bn_stats(
                out=acc[0 : 96 + 2 * G, 6 * r : 6 * (r + 1)],
                in_=sdt.pop(r)[0 : 96 + 2 * G, :],
            )

        stage_a(0)
        stage_a(1)
        stage_b(0)
        for bk in range(NBLK):
            stage_a(bk + 2)
            stage_b(bk + 1)
            stage_c(bk)
            for r in range(bk * BLK, min((bk + 1) * BLK, ROUNDS)):
                stage_d(r)
                stage_e(r - 1)
        stage_e(ROUNDS - 1)
        nc.sync.dma_start(out=out[0 : 96 + 2 * G, :], in_=acc[0 : 96 + 2 * G, :])
    nc.compile()
    return nc


def _get_nc():
    if "nc" not in _cached:
        _cached["nc"] = _build()
    return _cached["nc"]


def _shard(input, target):
    fp8 = ml_dtypes.float8_e4m3
    z = np.where(target > 0, -input, input)
    np.clip(z, -15.0, 4.4, out=z)
    z8 = z.astype(fp8).view(np.uint8)
    z8 = (z8 & np.uint8(0xFE)) | (target > 0).astype(np.uint8)
    bd = np.zeros((K, G), np.float32)
    for g in range(G):
        bd[g * C : (g + 1) * C, g] = 1.0
    zero = np.zeros_like(bd)
    w = np.concatenate([bd, zero, zero, bd], axis=1)  # w_s (+1|0), w_d (0|+1)
    w8 = w.astype(fp8).view(np.uint8)
    in_maps = []
    for cr in range(NCORES):
        zp = np.zeros((RPC, C), np.uint8)
        zp[:RPC_RAW] = z8[cr * RPC_RAW : (cr + 1) * RPC_RAW]
        zt = zp.reshape(G, M, C).transpose(0, 2, 1).reshape(K, M)
        in_maps.append({"z": np.ascontiguousarray(zt), "w": w8})
    return in_maps


_last_results = None


def kernel(input, target):
    global _last_results
    input = np.ascontiguousarray(np.asarray(input, dtype=np.float32))
    target = np.ascontiguousarray(np.asarray(target, dtype=np.int32))
    assert input.shape == (B, C) and target.shape == (B, C)

    from concourse.bass_utils import run_bass_kernel_spmd

    nc = _get_nc()
    in_maps = _shard(input, target)
    res = run_bass_kernel_spmd(nc, in_maps, core_ids=list(range(NCORES)))
    _last_results = res

    sign = np.zeros(128, np.float64)
    for s in range(RPS):
        sign[32 * s : 32 * s + G] = 1.0
        sign[32 * s + G : 32 * s + 2 * G] = -1.0

    total = 0.0
    for r in res.results:
        a = r["o"].astype(np.float64).reshape(128, ROUNDS, 6)
        sumsq = a[..., 2] + a[..., 0] * a[..., 1] ** 2 \
            + a[..., 5] + a[..., 3] * a[..., 4] ** 2
        total += float(np.einsum("p,pr->", sign, sumsq))
    total /= 4.0
    return np.asarray(np.log1p(total) / B, dtype=np.float32)
